# revision 9
# baseline (speedup 1.0000x reference)
"""GQA sliding-window attention (training path, no causal mask, no 1/sqrt(d)
scaling) on 8 Trainium2 NeuronCores.

Reference semantics (see original nn.Module):
  q = x@Wq+bq [b,s,16,64]; k,v = x@Wk+bk / x@Wv+bv [b,s,2,64]
  k,v zero-padded by 128 on both sides of s; query i attends padded
  positions [i, i+256); padded positions contribute score 0 (exp->1)
  and value 0. out = attn @ Wo + bo.

Sharding: batch x sequence. 8 shards = 2 batches x 4 chunks of 512 query
rows. Each core receives x^T for its 512 rows plus a 128-row halo on each
side (zero rows outside [0, 2048)), with an appended 0/1 validity row so
that K/V bias is only added at in-range positions. Host gathers per-core
outputs; no collectives.

Datapath: fp16 inputs for x/Wq/Wk/Wv (10-bit mantissa keeps q/k/score
precision at tf32 level), bf16 for probs/V/attnT/Wo (exp outputs exceed
fp16 range). All matmuls run 1 cycle/column on the PE; PSUM accumulation
stays fp32.

Per-core dataflow:
  K/V projections accumulate per 128-row contraction chunk as x^T chunks
  stream in, so the PE starts ~1.3us after launch. Q projection follows
  (wq loads during K/V). Attention uses exact 128x128 tiling: per head,
  4 q-tiles x 3 kv-chunks of S^T = kT^T qT scores (12 matmuls, N=128),
  one exp per half-head, band masking via a single precomputed triangle
  mask applied on the DVE, then 12 PV matmuls accumulating [65, 512]
  (row 64 = softmax denominator via a ones-column in V). Normalization
  (reciprocal + PE broadcast + DVE multiply) is interleaved in waves, then
  the Wo projection streams out.
"""

import numpy as np

DIM = 1024
NH = 16  # query heads
G = 2  # kv heads
HD = 64  # head dim
W = 256  # window
HALF = 128
BATCH, SEQ = 2, 2048
NCORES = 8
SQ = 512  # query rows per core
SK = SQ + 2 * HALF  # 768 kv halo rows per core
KC = DIM // 128  # 8 contraction chunks
NJ = SK // 128  # 6 kv chunks
NT = SQ // 128  # 4 q tiles

_CACHE = {}


def _build_program(dbg=False):
    import concourse.bass as bass
    import concourse.mybir as mybir
    import concourse.tile as tile
    from concourse import bacc

    f32 = mybir.dt.float32
    f32r = mybir.dt.float32r
    f16 = mybir.dt.float16
    bf16 = mybir.dt.bfloat16

    nc = bacc.Bacc("TRN2", target_bir_lowering=False, debug=False, num_devices=NCORES)
    dbg_t = {}
    if dbg:
        for name, shape, dt in [
            ("dbg_qT", [128, KC, SQ], f16), ("dbg_kT", [128, SK], f16),
            ("dbg_vt", [128, NJ, G, HD + 1], bf16),
            ("dbg_pt", [128, NH, 2, 2, 3, 128], bf16),
            ("dbg_attnP", [128, KC, SQ], f32r),
            ("dbg_y", [128, KC, SQ], f32),
            ("dbg_attnT", [128, KC, SQ], f32r),
            ("dbg_denr", [128, 3, SQ], f32r),
            ("dbg_den", [128, 3, SQ], f32),
        ]:
            dbg_t[name] = nc.declare_dram_parameter(name, shape, dt, isOutput=True)

    xaT = nc.declare_dram_parameter("xaT", [DIM + 1, SK], f16, isOutput=False)
    wq = nc.declare_dram_parameter("wq", [DIM, DIM], f16, isOutput=False)
    wk = nc.declare_dram_parameter("wk", [DIM + 1, G * HD], f16, isOutput=False)
    wv = nc.declare_dram_parameter("wv", [DIM + 1, G * HD], f16, isOutput=False)
    wo = nc.declare_dram_parameter("wo", [DIM, DIM], f32r, isOutput=False)
    bq = nc.declare_dram_parameter("bq", [DIM, 1], f32, isOutput=False)
    bo = nc.declare_dram_parameter("bo", [DIM, 1], f32, isOutput=False)
    sel2 = nc.declare_dram_parameter("sel2", [128, 128], f32r, isOutput=False)
    identD = nc.declare_dram_parameter("ident", [128, 128], bf16, isOutput=False)
    ones2 = nc.declare_dram_parameter("ones2", [128, G], bf16, isOutput=False)
    maskD = nc.declare_dram_parameter("maskD", [128, 2, 3, 128], bf16, isOutput=False)
    yT = nc.declare_dram_parameter("yT", [DIM, SQ], f32, isOutput=True)

    with tile.TileContext(nc) as tc:
        with (
            nc.allow_low_precision("fp16/bf16 matmul inputs; accumulation stays fp32"),
            tc.tile_pool(name="wts", bufs=1) as wts,
            tc.tile_pool(name="sb", bufs=1) as sb,
            tc.tile_pool(name="pt", bufs=3) as ptp,
            tc.tile_pool(name="yst", bufs=2) as yst,
            tc.tile_pool(name="big2", bufs=2, space="PSUM") as big2,
            tc.tile_pool(name="one", bufs=2, space="PSUM") as one,
            tc.tile_pool(name="pvP", bufs=2, space="PSUM") as pvP,
        ):
            # ---- constant loads ----
            # K/V weights + aug rows land first so the per-chunk K/V
            # accumulation starts as soon as xT chunk 0 arrives. wq follows
            # xT on the same queues; wo and the small constants ride the
            # GPSIMD SWDGE queue (all needed much later).
            wk_sb = wts.tile([128, KC, G * HD], f16, tag="wk")
            wv_sb = wts.tile([128, KC, G * HD], f16, tag="wv")
            xaug = wts.tile([1, SK], f16, tag="xaug")
            wk_aug = wts.tile([1, G * HD], f16, tag="wkaug")
            wv_aug = wts.tile([1, G * HD], f16, tag="wvaug")
            xT_sb = wts.tile([128, KC, SK], f16, tag="xT")
            wq_sb = wts.tile([128, KC, DIM], f16, tag="wq")
            wo_sb = wts.tile([128, KC, DIM], f32r, tag="wo")

            nc.sync.dma_start(
                out=wk_sb[:, :, :],
                in_=wk[0:DIM, :].rearrange("(a p) c -> p a c", p=128))
            nc.sync.dma_start(out=xaug[:, :], in_=xaT[DIM:DIM + 1, :])
            nc.scalar.dma_start(
                out=wv_sb[:, :, :],
                in_=wv[0:DIM, :].rearrange("(a p) c -> p a c", p=128))
            nc.scalar.dma_start(out=wk_aug[:, :], in_=wk[DIM:DIM + 1, :])
            nc.scalar.dma_start(out=wv_aug[:, :], in_=wv[DIM:DIM + 1, :])
            for kc in range(KC):
                eng = nc.sync if kc % 2 == 0 else nc.scalar
                eng.dma_start(out=xT_sb[:, kc, :], in_=xaT[kc * 128:(kc + 1) * 128, :])
            for kc in range(KC):
                eng = nc.sync if kc % 2 == 0 else nc.scalar
                eng.dma_start(out=wq_sb[:, kc, :], in_=wq[kc * 128:(kc + 1) * 128, :])

            bq_sb = wts.tile([128, KC], f32, tag="bq")
            bo_sb = wts.tile([128, KC], f32, tag="bo")
            sel2_sb = wts.tile([128, 128], f32r, tag="sel2")
            ident = wts.tile([128, 128], bf16, tag="ident")
            ones_sb = wts.tile([128, G], bf16, tag="ones")
            mask_sb = wts.tile([128, 2, 3, 128], bf16, tag="mask")
            nc.gpsimd.dma_start(
                out=bq_sb[:, :], in_=bq.rearrange("(a p) c -> p (a c)", p=128))
            nc.gpsimd.dma_start(
                out=bo_sb[:, :], in_=bo.rearrange("(a p) c -> p (a c)", p=128))
            nc.gpsimd.dma_start(out=sel2_sb[:, :], in_=sel2[:, :])
            nc.gpsimd.dma_start(out=ident[:, :], in_=identD[:, :])
            nc.gpsimd.dma_start(out=ones_sb[:, :], in_=ones2[:, :])
            nc.gpsimd.dma_start(out=mask_sb[:, :, :, :], in_=maskD[:, :, :, :])
            for kc in range(KC):
                nc.gpsimd.dma_start(out=wo_sb[:, kc, :], in_=wo[kc * 128:(kc + 1) * 128, :])

            # ---- persistent intermediates ----
            qT_sb = sb.tile([128, KC, SQ], f16, tag="qT")  # [dk(2 heads), dd, q]
            kT_sb = sb.tile([128, SK], f16, tag="kT")      # [dk(2 groups), w]
            vT_sb = sb.tile([128, SK], bf16, tag="vT")
            vt_t = [
                sb.tile([128, G, HD + 1], bf16, tag=f"vt{j}", name=f"vt{j}")
                for j in range(NJ)
            ]
            attnT = sb.tile([128, KC, SQ], f32r, tag="attnT")  # [dk(2 heads), pair, q]
            den = sb.tile([128, 3, SQ], f32, tag="den")
            den_r = sb.tile([128, 3, SQ], f32r, tag="denr")
            nc.vector.memset(den[:, :, :], 1.0)

            # ---- K/V projections, accumulated per contraction chunk ----
            tK = big2.tile([128, 2, 512], f32, tag="big", name="tK")
            tV = big2.tile([128, 2, 512], f32, tag="big", name="tV")
            for kc in range(KC):
                for h2 in range(2):
                    sl = slice(h2 * 384, (h2 + 1) * 384)
                    nc.tensor.matmul(tK[:, h2, 0:384], wk_sb[:, kc, :],
                                     xT_sb[:, kc, sl], start=(kc == 0), stop=False)
                    nc.tensor.matmul(tV[:, h2, 0:384], wv_sb[:, kc, :],
                                     xT_sb[:, kc, sl], start=(kc == 0), stop=False)
            for h2 in range(2):
                sl = slice(h2 * 384, (h2 + 1) * 384)
                nc.tensor.matmul(tK[:, h2, 0:384], wk_aug[:, :], xaug[:, sl],
                                 start=False, stop=True)
                nc.tensor.matmul(tV[:, h2, 0:384], wv_aug[:, :], xaug[:, sl],
                                 start=False, stop=True)
            nc.vector.tensor_copy(
                kT_sb.rearrange("p (a i) -> p a i", a=2), tK[:, :, 0:384])
            nc.vector.tensor_copy(
                vT_sb.rearrange("p (a i) -> p a i", a=2), tV[:, :, 0:384])

            # ---- V back to natural layout [w, dk], ones column appended ----
            for j in range(NJ):
                po = pvP.tile([128, 128], bf16, tag="pv", name=f"pstr{j}")
                nc.tensor.transpose(po, vT_sb[:, j * 128:(j + 1) * 128], ident)
                nc.vector.tensor_copy(
                    vt_t[j][:, :, 0:HD], po.rearrange("p (g d) -> p g d", g=G))
                nc.vector.tensor_copy(vt_t[j][:, :, HD:HD + 1], ones_sb[:, :])

            # ---- Q projection ----
            for dd in range(KC):
                ps = one.tile([128, 512], f32, tag="one", name=f"psq{dd}")
                for kc in range(KC):
                    nc.tensor.matmul(
                        ps, wq_sb[:, kc, dd * 128:(dd + 1) * 128],
                        xT_sb[:, kc, HALF:HALF + SQ],
                        start=(kc == 0), stop=(kc == KC - 1),
                    )
                nc.scalar.activation(
                    qT_sb[:, dd, :], ps, mybir.ActivationFunctionType.Identity,
                    bias=bq_sb[:, dd:dd + 1],
                )

            # ---- attention, normalization interleaved per wave ----
            # Host permutes Wq columns so q dd-block p holds head p (group 0)
            # in rows 0:64 and head p+8 (group 1) in rows 64:128 — score
            # matmul operands then share a base partition with kT's groups.
            def norm_recip(c3):
                nc.vector.reciprocal(den_r[:, c3, :], den[:, c3, :])

            def norm_apply(plo, phi):
                for p in range(plo, phi):
                    ps = one.tile([128, 512], f32, tag="one", name=f"psn{p}")
                    k4 = 32 * (p % 3)
                    nc.tensor.matmul(ps, sel2_sb[k4:k4 + 2, :],
                                     den_r[k4:k4 + 2, p // 3, :],
                                     start=True, stop=True)
                    nc.vector.tensor_mul(attnT[:, p, :], attnT[:, p, :], ps)

            for p_g in [(p, gg) for p in range(KC) for gg in range(G)]:
                p, gg = p_g
                if (p, gg) == (3, 0):
                    norm_recip(0)
                if (p, gg) == (5, 0):
                    norm_apply(0, 3)
                if (p, gg) == (6, 0):
                    norm_recip(1)
                if (p, gg) == (7, 0):
                    nc.vector.reciprocal(den_r[0:2, 2, :], den[0:2, 2, :])
                if (p, gg) == (7, 1):
                    norm_apply(3, 7)
                h = p + 8 * gg
                g = gg
                qrow = 64 * gg
                qT_h = qT_sb[qrow:qrow + 64, p, :]
                kT_g = kT_sb[64 * g:64 * g + 64, :]
                pt = ptp.tile([128, 2, 2, 3, 128], bf16, tag="pt", name=f"pt{h}")
                for hh in range(2):
                    psc = big2.tile([128, 2, 512], f32, tag="big", name=f"psc{h}_{hh}")
                    # scores S^T per (q-tile t, kv-chunk c): kv chunk j=t+c.
                    # j-major order shares the kT stationary between tiles.
                    for j in range(2 * hh, 2 * hh + 4):
                        for t in (2 * hh, 2 * hh + 1):
                            c = j - t
                            if 0 <= c <= 2:
                                nc.tensor.matmul(
                                    psc[:, t % 2, 128 * c:128 * c + 128],
                                    kT_g[:, 128 * j:128 * j + 128],
                                    qT_h[:, 128 * t:128 * t + 128],
                                    start=True, stop=True,
                                )
                    nc.scalar.activation(
                        pt[:, hh], psc[:, :, 0:384].rearrange("p a (c i) -> p a c i", c=3),
                        mybir.ActivationFunctionType.Exp)
                    # band mask: c=0 keeps kv_row >= q_col, c=1 all, c=2 <
                    nc.vector.tensor_mul(pt[:, hh], pt[:, hh], mask_sb[:, :, :, :])
                # ONE start per pv bank: start=True clears has_written for the
                # whole 2KB zero region, so later region-starts must rely on
                # fresh-byte overwrite semantics instead of their own start.
                pv = pvP.tile([128, 512], f32, tag="pv", name=f"pv{h}")
                for hh in range(2):
                    for j in range(2 * hh, 2 * hh + 4):
                        for t in (2 * hh, 2 * hh + 1):
                            c = j - t
                            if 0 <= c <= 2:
                                nc.tensor.matmul(
                                    pv[0:HD + 1, 128 * t:128 * t + 128],
                                    vt_t[j][:, g, :],
                                    pt[:, hh, t % 2, c, :],
                                    start=(hh == 0 and j == 0),
                                    stop=(hh == 1 and j == 5),
                                )
                if dbg:
                    nc.sync.dma_start(out=dbg_t["dbg_pt"][:, h, :, :, :, :], in_=pt[:, :, :, :, :])
                nc.vector.tensor_copy(attnT[qrow:qrow + 64, p, :], pv[0:HD, :])
                if dbg:
                    nc.sync.dma_start(out=dbg_t["dbg_attnP"][qrow:qrow + 64, p, :], in_=attnT[qrow:qrow + 64, p, :])
                if gg == 0:
                    nc.vector.tensor_copy(
                        den[32 * (p % 3):32 * (p % 3) + 1, p // 3, :], pv[HD:HD + 1, :])
                else:
                    # engine writes must start at partition 0/32/64/96; bounce
                    # through partition 0 and DMA into den partition 32k+1
                    dtmp = yst.tile([1, SQ], f32, tag="dtmp", name=f"dtmp{h}")
                    nc.vector.tensor_copy(dtmp[:, :], pv[HD:HD + 1, :])
                    nc.sync.dma_start(
                        out=den[32 * (p % 3) + 1:32 * (p % 3) + 2, p // 3, :],
                        in_=dtmp[:, :])

            nc.vector.reciprocal(den_r[32:34, 2, :], den[32:34, 2, :])
            norm_apply(7, 8)

            if dbg:
                nc.sync.dma_start(out=dbg_t["dbg_qT"][:, :, :], in_=qT_sb[:, :, :])
                nc.sync.dma_start(out=dbg_t["dbg_kT"][:, :], in_=kT_sb[:, :])
                for j in range(NJ):
                    nc.sync.dma_start(out=dbg_t["dbg_vt"][:, j, :, :], in_=vt_t[j][:, :, :])
                nc.sync.dma_start(out=dbg_t["dbg_attnT"][:, :, :], in_=attnT[:, :, :])
                nc.sync.dma_start(out=dbg_t["dbg_den"][:, :, :], in_=den[:, :, :])
                nc.sync.dma_start(out=dbg_t["dbg_denr"][:, :, :], in_=den_r[:, :, :])

            # ---- output projection ----
            for do in range(KC):
                ps = one.tile([128, 512], f32, tag="one", name=f"pso{do}")
                for p in range(KC):
                    nc.tensor.matmul(
                        ps, wo_sb[:, p, do * 128:(do + 1) * 128], attnT[:, p, :],
                        start=(p == 0), stop=(p == KC - 1),
                    )
                yt = yst.tile([128, SQ], f32, tag="yt", name=f"yt{do}")
                nc.scalar.activation(yt, ps, mybir.ActivationFunctionType.Identity,
                                     bias=bo_sb[:, do:do + 1])
                eng = nc.sync if do % 2 == 0 else nc.scalar
                eng.dma_start(out=yT[do * 128:(do + 1) * 128, :], in_=yt[:, :])

    nc.finalize()
    return nc


def get_program():
    if "nc" not in _CACHE:
        _CACHE["nc"] = _build_program()
    return _CACHE["nc"]


def make_in_maps(x, Wq, bq, Wk, bk, Wv, bv, Wo, bo):
    """Host-side sharding: per-core input dicts."""
    import ml_dtypes

    bf = ml_dtypes.bfloat16
    x = np.ascontiguousarray(np.asarray(x, np.float32))
    wkb = np.concatenate([np.asarray(Wk, np.float32), np.asarray(bk, np.float32)[None]], 0)
    wvb = np.concatenate([np.asarray(Wv, np.float32), np.asarray(bv, np.float32)[None]], 0)
    sel2 = np.zeros((128, 128), np.float32)
    sel2[0::32, :64] = 1.0
    sel2[1::32, 64:] = 1.0
    # head permutation: device column-block p holds [head p | head p+8]
    perm = np.empty(DIM, np.int64)
    for p in range(8):
        perm[128 * p:128 * p + 64] = np.arange(64 * p, 64 * p + 64)
        perm[128 * p + 64:128 * p + 128] = np.arange(64 * (p + 8), 64 * (p + 8) + 64)
    # triangle band mask per (tile, chunk) block: c=0 keep p>=i, c=1 all,
    # c=2 keep p<i (identical for every q-tile and head)
    pi = np.arange(128)
    mask = np.ones((128, 2, 3, 128), np.float32)
    mask[:, :, 0, :] = (pi[:, None] >= pi[None, :])[:, None, :]
    mask[:, :, 2, :] = (pi[:, None] < pi[None, :])[:, None, :]
    common = {
        "wq": np.ascontiguousarray(np.asarray(Wq, np.float32)[:, perm]).astype(np.float16),
        "wk": np.ascontiguousarray(wkb).astype(np.float16),
        "wv": np.ascontiguousarray(wvb).astype(np.float16),
        "wo": np.ascontiguousarray(np.asarray(Wo, np.float32)[perm, :]),
        "bq": np.ascontiguousarray(np.asarray(bq, np.float32)[perm].reshape(DIM, 1)),
        "bo": np.ascontiguousarray(np.asarray(bo, np.float32).reshape(DIM, 1)),
        "sel2": sel2,
        "ident": np.eye(128, dtype=np.float32).astype(bf),
        "ones2": np.ones((128, G), np.float32).astype(bf),
        "maskD": mask.astype(bf),
    }
    in_maps = []
    for c in range(NCORES):
        b, t = divmod(c, NCORES // BATCH)
        s0 = SQ * t
        xa = np.zeros((SK, DIM + 1), np.float32)
        lo, hi = max(0, s0 - HALF), min(SEQ, s0 + SQ + HALF)
        xa[lo - (s0 - HALF):hi - (s0 - HALF), :DIM] = x[b, lo:hi]
        xa[lo - (s0 - HALF):hi - (s0 - HALF), DIM] = 1.0
        in_maps.append({"xaT": np.ascontiguousarray(xa.T).astype(np.float16), **common})
    return in_maps


def assemble_output(results):
    y = np.empty((BATCH, SEQ, DIM), np.float32)
    for c in range(NCORES):
        b, t = divmod(c, NCORES // BATCH)
        y[b, SQ * t:SQ * (t + 1), :] = results[c]["yT"].T
    return y


def kernel(**inputs):
    from concourse.bass_utils import run_bass_kernel_spmd

    nc = get_program()
    in_maps = make_in_maps(**inputs)
    last_err = None
    for _ in range(3):  # retry: transient NRT device wedges recover on rerun
        try:
            res = run_bass_kernel_spmd(nc, in_maps, list(range(NCORES)))
            return assemble_output(res.results)
        except Exception as e:  # noqa: BLE001
            last_err = e
    raise last_err
